# revision 1
# baseline (speedup 1.0000x reference)
"""Trainium2 Bass kernel for nn_BiRNNImputerModel (bidirectional GRU imputer).

Strategy:
  - 8 cores: cores 0-3 run the forward GRU, cores 4-7 the backward GRU
    (backward = same program on time-reversed inputs).
  - Within a direction, data-parallel over batch: 128 / 4 = 32 per core.
  - Everything on-chip lives in "transposed" layout [feature/H, batch] so the
    recurrent matmuls need no per-step transposes:
        gates^T[3H, B] = W^T-stationary @ activations-moving.
  - Weights & moving operands in bf16 (fp32 PSUM accumulation), gate math in
    fp32 from PSUM.
  - The per-step readout matmul uses a stacked stationary [Wro.T | WoutX.T]
    so each step produces xhat_t (imputer readout, rows 0:64) AND this
    direction's partial of the final bidirectional readout (rows 64:128)
    in one accumulation — no separate tail GEMM, h needs only a 2-step ring.
  - The cross-direction sum + bout + layout fixes happen on the host (cheap
    numpy); there is no cross-core communication at all.

PSUM discipline: start=True clears has_written BANK-wide, so each psum bank
gets exactly ONE start=True per step (its first write); all other matmuls use
start=False and rely on per-element overwrite-where-clear semantics.
"""

import os
import sys

for _p in ("/opt/trn_rl_repo", "/root/.axon_site/_ro/trn_rl_repo"):
    if os.path.isdir(_p) and _p not in sys.path:
        sys.path.insert(0, _p)

import numpy as np
import ml_dtypes

import concourse.bass as bass
import concourse.tile as tile
from concourse import mybir
from concourse.bass_utils import run_bass_kernel_spmd

BF16 = ml_dtypes.bfloat16

B, S, N, C = 128, 512, 64, 1
F = N * C          # 64
H = 512
NB = 32            # batch per core (128 / 4)
NFOLD = 4          # H / 128
AF = mybir.ActivationFunctionType
ALU = mybir.AluOpType


def _legalize_multiwait(nc, max_waits=1):
    """walrus in this image only encodes one sync-wait per instruction;
    hoist extra waits onto preceding NoOps."""
    n_fix = 0
    for f in nc.m.functions:
        for blk in f.blocks:
            new = []
            for ins in blk.instructions:
                si = getattr(ins, "sync_info", None)
                if si is not None and si.on_wait and len(si.on_wait) > max_waits:
                    waits = list(si.on_wait)
                    si.on_wait = waits[-max_waits:]
                    for i, w in enumerate(waits[:-max_waits]):
                        new.append(
                            mybir.InstNoOp(
                                name=f"{ins.name}-waitfix-{i}",
                                engine=ins.engine,
                                sync_info=mybir.SyncInfo(on_wait=[w], on_update=[]),
                                bass_nofuse=True,
                            )
                        )
                        n_fix += 1
                new.append(ins)
            blk.instructions[:] = new
    return n_fix


def build_nc(n_steps):
    """Build the per-core SPMD program. n_steps = S - 1 recurrent steps."""
    nc = bass.Bass()
    dt = mybir.dt

    xm = nc.dram_tensor("xm", [F, n_steps, 2 * NB], dt.bfloat16, kind="ExternalInput")
    wih = nc.dram_tensor("wih", [2 * F, 3 * H], dt.bfloat16, kind="ExternalInput")
    whh = nc.dram_tensor("whh", [128, NFOLD * 3 * H], dt.bfloat16, kind="ExternalInput")
    # stacked readout: fold c -> [Wro.T fold | WoutX.T fold] = [128, 128]
    wro = nc.dram_tensor("wro", [128, NFOLD * 128], dt.bfloat16, kind="ExternalInput")
    brz = nc.dram_tensor("brz", [128, 8], dt.float32, kind="ExternalInput")
    bin_ = nc.dram_tensor("bin", [128, NFOLD], dt.float32, kind="ExternalInput")
    bhn = nc.dram_tensor("bhn", [128, NFOLD], dt.float32, kind="ExternalInput")
    bro = nc.dram_tensor("bro", [F, 1], dt.float32, kind="ExternalInput")

    op_out = nc.dram_tensor("op", [128, n_steps, NB], dt.float32, kind="ExternalOutput")

    with tile.TileContext(nc) as tc:
        with (
            tc.tile_pool(name="singles", bufs=1) as singles,
            tc.tile_pool(name="hist", bufs=1) as hist,
            tc.tile_pool(name="xin", bufs=4) as xinp,
            tc.tile_pool(name="xtp", bufs=4) as xtp,
            tc.tile_pool(name="work", bufs=3) as work,
            tc.tile_pool(name="ps", bufs=1, space="PSUM") as psp,
            tc.tile_pool(name="psro", bufs=3, space="PSUM") as psro,
            tc.tile_pool(name="outs", bufs=3) as outs,
        ):
            # --- load weights / biases (once) ---
            wih_sb = singles.tile([2 * F, 3 * H], dt.bfloat16)
            nc.sync.dma_start(out=wih_sb, in_=wih[:])
            whh_sb = singles.tile([128, NFOLD * 3 * H], dt.bfloat16)
            nc.sync.dma_start(out=whh_sb, in_=whh[:])
            wro_sb = singles.tile([128, NFOLD * 128], dt.bfloat16)
            nc.sync.dma_start(out=wro_sb, in_=wro[:])
            brz_sb = singles.tile([128, 8], dt.float32)
            nc.sync.dma_start(out=brz_sb, in_=brz[:])
            bin_sb = singles.tile([128, NFOLD], dt.float32)
            nc.sync.dma_start(out=bin_sb, in_=bin_[:])
            bhn_sb = singles.tile([128, NFOLD], dt.float32)
            nc.sync.dma_start(out=bhn_sb, in_=bhn[:])
            bro_sb = singles.tile([F, 1], dt.float32)
            nc.sync.dma_start(out=bro_sb, in_=bro[:])

            # hidden state ring: [128, fold, parity, batch]; parity = t % 2
            h_hist = hist.tile([128, NFOLD, 2, NB], dt.bfloat16)
            nc.vector.memset(h_hist[:, :, 0, :], 0.0)

            # x_in for step 1: rows 0:64 <- bro (xhat_0), rows 64:128 <- m_0
            x_in = xinp.tile([2 * F, NB], dt.bfloat16)
            nc.sync.dma_start(out=x_in[F : 2 * F, :], in_=xm[:, 0, NB : 2 * NB])
            nc.vector.memset(x_in[0:F, :], 0.0)
            nc.scalar.activation(
                out=x_in[0:F, :], in_=x_in[0:F, :], func=AF.Identity,
                bias=bro_sb[:, 0:1], scale=1.0,
            )
            xm_t = xtp.tile([F, 2 * NB], dt.bfloat16, tag="xm_t", name="xmt0")
            nc.sync.dma_start(out=xm_t, in_=xm[:, 0, :])
            nc.vector.copy_predicated(
                x_in[0:F, :], xm_t[:, NB : 2 * NB].bitcast(mybir.dt.uint16),
                xm_t[:, 0:NB],
            )

            prev = None  # deferred readout state: (ps_ro, hnews, tp)

            def emit_readout_tail(prev, build_xin):
                ps_ro_p, hnews, tp = prev
                for k, g in enumerate((0, 2, 1, 3)):
                    nc.tensor.matmul(ps_ro_p, wro_sb[:, g * 128 : (g + 1) * 128],
                                     hnews[g], start=(k == 0), stop=(k == 3))
                out_t = outs.tile([128, NB], dt.float32, tag="out_t", name=f"out{tp}")
                nc.vector.tensor_scalar_add(out_t[0:F, :], ps_ro_p[0:F, :],
                                            bro_sb[:, 0:1])
                nc.scalar.activation(out=out_t[F : 2 * F, :], in_=ps_ro_p[F : 2 * F, :],
                                     func=AF.Copy)
                nc.sync.dma_start(out=op_out[:, tp - 1, :], in_=out_t)
                if not build_xin:
                    return None
                x_in_n = xinp.tile([2 * F, NB], dt.bfloat16, name=f"xin{tp}")
                nc.sync.dma_start(out=x_in_n[F : 2 * F, :], in_=xm[:, tp, NB : 2 * NB])
                nc.vector.tensor_scalar_add(x_in_n[0:F, :], ps_ro_p[0:F, :],
                                            bro_sb[:, 0:1])
                xm_t = xtp.tile([F, 2 * NB], dt.bfloat16, tag="xm_t", name=f"xmt{tp}")
                nc.sync.dma_start(out=xm_t, in_=xm[:, tp, :])
                nc.vector.copy_predicated(
                    x_in_n[0:F, :], xm_t[:, NB : 2 * NB].bitcast(mybir.dt.uint16),
                    xm_t[:, 0:NB],
                )
                return x_in_n

            for t in range(1, n_steps + 1):
                pv, cur = (t - 1) % 2, t % 2
                hprev = lambda c: h_hist[:, c, pv, :]
                ps_ro = psro.tile([128, NB], dt.float32, tag="ps_ro")
                r_t = work.tile([128, NFOLD * NB], dt.bfloat16, tag="r_t")
                z_t = work.tile([128, NFOLD * NB], dt.bfloat16, tag="z_t")
                n_t = work.tile([128, NFOLD * NB], dt.bfloat16, tag="n_t")
                # one psum tile (= one bank) per fold-group:
                # regions: [0]=r, [1]=z, [2]=gi_n, [3]=gh_n
                ps_g = [psp.tile([128, 4 * NB], dt.float32, tag=f"ps{c}", name=f"psg{c}")
                        for c in range(NFOLD)]

                def reg(g, j):
                    return ps_g[g][:, j * NB : (j + 1) * NB]

                def whh_sl(c2, g):
                    return whh_sb[:, c2 * 3 * H + 128 * g : c2 * 3 * H + 128 * (g + 1)]

                started = set()

                def gh_mm(g, c2):
                    for j, gate in enumerate((g, 4 + g, 8 + g)):
                        st = (g, j) not in started and (g % 2 == 0 or (2 * (g // 2), j) in started or True)
                        nc.tensor.matmul(
                            reg(g, j if j < 2 else 3), whh_sl(c2, gate),
                            hprev(c2),
                            start=(j == 0 and c2 == 0),
                            stop=(j == 2 and c2 == NFOLD - 1),
                            skip_group_check=True,
                        )

                def gi_mm(g):
                    for j, gate in enumerate((g, 4 + g, 8 + g)):
                        wsl = slice(128 * gate, 128 * (gate + 1))
                        nc.tensor.matmul(reg(g, min(j, 2)), wih_sb[:, wsl], x_in,
                                         start=False, stop=True,
                                         skip_group_check=True)

                def gate_math(g):
                    gs = slice(g * NB, (g + 1) * NB)
                    nc.scalar.activation(out=r_t[:, gs], in_=reg(g, 0), func=AF.Sigmoid,
                                         bias=brz_sb[:, g : g + 1])
                    nc.scalar.activation(out=z_t[:, gs], in_=reg(g, 1), func=AF.Sigmoid,
                                         bias=brz_sb[:, 4 + g : 5 + g])
                    zh_c = work.tile([128, NB], dt.float32, tag="zh_c", name=f"zh{g}")
                    nc.gpsimd.tensor_tensor(zh_c, z_t[:, gs], hprev(g), ALU.mult)
                    omz_c = work.tile([128, NB], dt.bfloat16, tag="omz_c", name=f"om{g}")
                    nc.gpsimd.tensor_scalar(omz_c, z_t[:, gs], -1.0, 1.0,
                                            ALU.mult, ALU.add)
                    tmp_c = work.tile([128, NB], dt.float32, tag="tmp_c", name=f"tm{g}")
                    nc.vector.scalar_tensor_tensor(
                        out=tmp_c, in0=reg(g, 3), scalar=bhn_sb[:, g : g + 1],
                        in1=r_t[:, gs], op0=ALU.add, op1=ALU.mult,
                    )
                    nin_c = work.tile([128, NB], dt.float32, tag="nin_c", name=f"ni{g}")
                    nc.vector.tensor_tensor(nin_c, tmp_c, reg(g, 2), ALU.add)
                    nc.scalar.activation(out=n_t[:, gs], in_=nin_c, func=AF.Tanh,
                                         bias=bin_sb[:, g : g + 1])
                    t3_c = work.tile([128, NB], dt.float32, tag="t3_c", name=f"t3{g}")
                    nc.vector.tensor_tensor(t3_c, n_t[:, gs], omz_c, ALU.mult)
                    hnew_c = h_hist[:, g, cur, :]
                    nc.vector.tensor_tensor(hnew_c, t3_c, zh_c, ALU.add)
                    hnews[g] = hnew_c

                # Interleaved stream: close each fold-group's psum as early as
                # the fold-readiness of step t-1 allows, so gate chains start
                # early and overlap the rest of the matmul stream.
                hnews = [None] * NFOLD
                gh_mm(0, 0); gh_mm(0, 2); gh_mm(0, 1)
                gh_mm(2, 0); gh_mm(2, 2); gh_mm(2, 1)
                gh_mm(1, 0); gh_mm(1, 2); gh_mm(1, 1)
                if prev is not None:
                    x_in = emit_readout_tail(prev, build_xin=True)
                gh_mm(3, 0); gh_mm(3, 2); gh_mm(3, 1)
                gh_mm(0, 3); gi_mm(0); gate_math(0)
                gh_mm(2, 3); gi_mm(2); gate_math(2)
                gh_mm(1, 3); gi_mm(1); gate_math(1)
                gh_mm(3, 3); gi_mm(3); gate_math(3)

                prev = (ps_ro, hnews, t)

            emit_readout_tail(prev, build_xin=False)

    _legalize_multiwait(nc)
    return nc


_NC_CACHE = {}


def _get_nc(n_steps):
    if n_steps not in _NC_CACHE:
        _NC_CACHE[n_steps] = build_nc(n_steps)
    return _NC_CACHE[n_steps]


def _prep_core_inputs(x2d, m2d, Wih, Whh, bih, bhh, Wro, bro, Wout_half, n_steps):
    """Per-core input map. x2d/m2d: [NB, S_loc, F] float32/bool already
    direction-ordered (time-reversed for backward cores)."""
    xt = x2d[:, :n_steps].transpose(2, 1, 0)          # [F, t, NB]
    mt = m2d[:, :n_steps].transpose(2, 1, 0).astype(np.float32)
    xm = np.concatenate([xt, mt], axis=2).astype(BF16)  # [F, t, 2*NB]
    wih_t = np.ascontiguousarray(Wih.T).astype(BF16)                      # [128, 3H]
    whh_t = np.ascontiguousarray(
        Whh.T.reshape(NFOLD, 128, 3 * H).transpose(1, 0, 2).reshape(128, NFOLD * 3 * H)
    ).astype(BF16)
    # stacked readout weights: fold c -> [Wro.T fold | WoutX.T fold]
    wro_f = Wro.T.reshape(NFOLD, 128, F)
    wout_f = Wout_half.T.reshape(NFOLD, 128, F)
    wro_t = np.ascontiguousarray(
        np.concatenate([wro_f, wout_f], axis=2).transpose(1, 0, 2).reshape(128, NFOLD * 128)
    ).astype(BF16)
    bsum = bih + bhh
    brz = np.stack([bsum[128 * g : 128 * (g + 1)] for g in range(8)], axis=1).astype(np.float32)
    bin_ = np.stack([bih[1024 + 128 * c : 1024 + 128 * (c + 1)] for c in range(NFOLD)], axis=1).astype(np.float32)
    bhn = np.stack([bhh[1024 + 128 * c : 1024 + 128 * (c + 1)] for c in range(NFOLD)], axis=1).astype(np.float32)
    return {
        "xm": xm, "wih": wih_t, "whh": whh_t, "wro": wro_t,
        "brz": brz, "bin": bin_, "bhn": bhn,
        "bro": bro.reshape(F, 1).astype(np.float32),
    }


def run_device(inputs, s_len=S, trace=False):
    """Run the 8-core SPMD kernel. Returns BassKernelResults."""
    n_steps = s_len - 1
    nc = _get_nc(n_steps)

    x2d = np.asarray(inputs["x"], np.float32).reshape(B, S, F)[:, :s_len]
    m2d = np.asarray(inputs["mask"]).reshape(B, S, F)[:, :s_len]

    in_maps = []
    for core in range(8):
        g = core % 4
        bsl = slice(NB * g, NB * (g + 1))
        if core < 4:
            im = _prep_core_inputs(
                x2d[bsl], m2d[bsl], inputs["Wih_f"], inputs["Whh_f"],
                inputs["bih_f"], inputs["bhh_f"], inputs["Wro_f"], inputs["bro_f"],
                np.asarray(inputs["Wout"])[:, :H], n_steps,
            )
        else:
            im = _prep_core_inputs(
                x2d[bsl, ::-1], m2d[bsl, ::-1], inputs["Wih_b"], inputs["Whh_b"],
                inputs["bih_b"], inputs["bhh_b"], inputs["Wro_b"], inputs["bro_b"],
                np.asarray(inputs["Wout"])[:, H:], n_steps,
            )
        in_maps.append(im)

    return run_bass_kernel_spmd(nc, in_maps, core_ids=list(range(8)), trace=trace)


def assemble(inputs, res, s_len=S):
    """Host-side gather: combine per-core outputs into full reference outputs."""
    n_steps = s_len - 1
    bro_f = np.asarray(inputs["bro_f"], np.float32)
    bro_b = np.asarray(inputs["bro_b"], np.float32)
    bout = np.asarray(inputs["bout"], np.float32)

    xh_f = np.empty((B, s_len, F), np.float32)
    xh_b = np.empty((B, s_len, F), np.float32)
    x_hat = np.empty((B, s_len, F), np.float32)

    for g in range(4):
        bsl = slice(NB * g, NB * (g + 1))
        rf, rb = res.results[g], res.results[g + 4]
        # device output "op" is [128, n_steps, NB]: rows 0:64 xhat, 64:128 pp
        xh_f[bsl, 1:] = rf["op"][:F].transpose(2, 1, 0)
        xh_f[bsl, 0] = bro_f
        xh_b[bsl, :n_steps] = rb["op"][:F].transpose(2, 1, 0)[:, ::-1]
        xh_b[bsl, n_steps] = bro_b
        pf = rf["op"][F:].transpose(2, 1, 0)
        pb = rb["op"][F:].transpose(2, 1, 0)[:, ::-1]
        x_hat[bsl, 1:] = pf
        x_hat[bsl, 0] = 0.0
        x_hat[bsl, :n_steps] += pb
        x_hat[bsl] += bout

    return (
        x_hat.reshape(B, s_len, N, C),
        xh_f.reshape(B, s_len, N, C),
        xh_b.reshape(B, s_len, N, C),
    )


def kernel(**inputs):
    res = run_device(inputs, s_len=S)
    return assemble(inputs, res, s_len=S)

